# revision 28
# baseline (speedup 1.0000x reference)
"""Dechirp-STFT Trainium2 kernel.

Computes, for each of D=16 chirp hypotheses, a resampled (linear-interp)
version of each Hann-windowed signal frame followed by a 1024-point FFT.

Strategy
--------
Per chirp d the whole frame-wise operation (gather/lerp resample -> DFT) is a
single linear map on the 1024-sample frame, so we fold both into one dense
matrix M_d built on the host from `dlnf`:  X_d = frames @ M_d.
Only rFFT bins f=0..512 are computed on device (input frames are real, so
the upper half is the conjugate mirror, filled in on the host). Device rows
use the packed-rfft column order [re0, re1, im1, ..., re511, im511, re512]
(1024 cols; im0/im512 are identically zero and filled host-side), which makes
every matmul an exact 512-wide PSUM bank and every output row 4096B-aligned.

Sharding: D axis across the 8 NeuronCores (2 chirps per core). Every core
holds the full transposed frame matrix wT (1024 x 2048, frame count padded
2047->2048) and its two combined matrices (1024 x 1024 each). The device
kernel is a plain tiled matmul: out[c] = wT.T @ M_c, 128x512 output tiles,
K=1024 contraction (8 k-tiles), PSUM accumulation, DVE copyback, DMA out.

Dataflow details (from trace analysis):
- inputs are stored partition-major in DRAM and loaded as k-quarter pairs
  in lockstep across the Sync and Scalar HWDGE rings; a short warmup burst
  un-throttles the PE clock (HAM -> 2.4 GHz), then 4 partial-K accumulation
  groups chase the quarter arrivals so real matmuls overlap the input load;
- outputs are staged GRP=4 row-tiles at a time and written with 16KB
  per-partition descriptors on the Sync ring (partition-major DRAM layout,
  untangled on the host);
- dtype mode "fp16" (default) runs the PE at full 16-bit rate with 10
  mantissa bits of input precision (values here are O(1), well within range).
  Measured ~134.6us HW time, ~2.8e-4 relative error.
"""

import os
import numpy as np

K = 1024
HOP = 512
NW = 2047          # (1048576 - 1024) // 512 + 1
RWS = 2048         # padded row count (16 x 128 tiles)
NRT = RWS // 128   # 16 row tiles
GRP = 4            # row tiles per output staging group
NG = NRT // GRP
D = 16
NCORES = 8
DPC = D // NCORES  # chirps per core
NF = K // 2 + 1    # 513 rfft bins
# device computes 1024 cols per row in packed-rfft order:
# [re0, re1, im1, re2, im2, ..., re511, im511, re512]
# (im0 and im512 are identically zero and are filled on the host)
OCOLS = 1024
CT = 512           # matmul col-tile (2 x 512 = 1024)
NCT = OCOLS // CT
KT = K // 128      # 8 contraction tiles

# device dtype mode: "bf16" | "fp16" | "f32" | "f32r"
DEV_DT = os.environ.get("BASS_KERNEL_DT", "fp16")

_NC_CACHE = {}


def _build_pos(dlnf):
    """lo/frac per chirp, replicating the reference's fp32 op chain bit-exactly
    (jax-on-CPU); falls back to numpy fp32 if jax is unavailable."""
    try:
        import jax
        import jax.numpy as jnp

        with jax.default_device(jax.devices("cpu")[0]):
            betas = 2.0 * jnp.asarray(dlnf, dtype=jnp.float32)
            tau = jnp.linspace(0.0, 1.0, K)
            safe = jnp.abs(betas) < 1e-8
            betas_safe = jnp.where(safe, jnp.float32(1e-8), betas)
            eb = jnp.exp(betas_safe)
            t_source = 2.0 / betas_safe[:, None] * jnp.log1p(
                tau[None, :] * (eb[:, None] - 1.0)
            ) - 1.0
            identity = jnp.linspace(-1.0, 1.0, K)
            t_source = jnp.where(safe[:, None], identity[None, :], t_source)
            pos = np.asarray((t_source + 1.0) * 0.5 * (K - 1), dtype=np.float32)
            win = np.asarray(
                0.5 * (1.0 - jnp.cos(2.0 * jnp.pi * jnp.arange(K, dtype=jnp.float32) / K)),
                dtype=np.float32,
            )
    except Exception:
        d32 = np.asarray(dlnf, dtype=np.float32)
        betas = (np.float32(2.0) * d32).astype(np.float32)
        tau = np.linspace(0.0, 1.0, K, dtype=np.float32)
        safe = np.abs(betas) < np.float32(1e-8)
        betas_safe = np.where(safe, np.float32(1e-8), betas).astype(np.float32)
        eb = np.exp(betas_safe).astype(np.float32)
        t_source = (np.float32(2.0) / betas_safe)[:, None] * np.log1p(
            tau[None, :] * (eb[:, None] - np.float32(1.0))
        ).astype(np.float32) - np.float32(1.0)
        identity = np.linspace(-1.0, 1.0, K, dtype=np.float32)
        t_source = np.where(safe[:, None], identity[None, :], t_source).astype(np.float32)
        pos = ((t_source + np.float32(1.0)) * np.float32(0.5) * np.float32(K - 1)).astype(np.float32)
        n = np.arange(K, dtype=np.float32)
        win = (np.float32(0.5) * (np.float32(1.0) - np.cos(np.float32(2.0 * np.pi) * n / np.float32(K)))).astype(np.float32)

    lo = np.clip(pos.astype(np.int32), 0, K - 2)
    frac = (pos - lo.astype(np.float32)).astype(np.float32)
    return lo, frac, win


def _build_mats(dlnf):
    """(D, K, OCOLS) float32 combined interp+rDFT matrices, packed-rfft cols."""
    lo, frac, win = _build_pos(dlnf)
    n = np.arange(K, dtype=np.float64)
    f = np.arange(NF, dtype=np.float64)
    E = np.exp(-2j * np.pi * np.outer(n, f) / K)  # (K, NF) c128
    mats = np.empty((D, K, OCOLS), np.float32)
    for d in range(D):
        C = np.zeros((K, NF), np.complex128)
        np.add.at(C, lo[d], E * (1.0 - frac[d].astype(np.float64))[:, None])
        np.add.at(C, lo[d] + 1, E * frac[d].astype(np.float64)[:, None])
        mats[d, :, 0] = C.real[:, 0].astype(np.float32)
        mats[d, :, 1:-1:2] = C.real[:, 1:-1].astype(np.float32)
        mats[d, :, 2:-1:2] = C.imag[:, 1:-1].astype(np.float32)
        mats[d, :, -1] = C.real[:, -1].astype(np.float32)
    return mats, win


def _dtypes(dt_key):
    import concourse.mybir as mybir

    return {
        "bf16": (mybir.dt.bfloat16, mybir.dt.bfloat16),
        "f32": (mybir.dt.float32, mybir.dt.float32),
        "f32r": (mybir.dt.float32r, mybir.dt.float32r),
        "fp16": (mybir.dt.float16, mybir.dt.float16),
    }[dt_key]


def _build_nc(dt_key):
    import concourse.mybir as mybir
    from concourse import bacc
    from concourse.tile import TileContext

    dt_w, dt_m = _dtypes(dt_key)

    nc = bacc.Bacc(
        "TRN2", target_bir_lowering=False, debug=False, num_devices=NCORES
    )
    # partition-major layouts: dram[p, k, :] belongs to SBUF partition p
    wT = nc.declare_dram_parameter("wT", [128, KT, RWS], dt_w, isOutput=False)
    mats = nc.declare_dram_parameter("mats", [DPC, 128, KT, OCOLS], dt_m, isOutput=False)
    # output partition-major: out[c, p, t, :] = result row t*128+p of chirp c
    out = nc.declare_dram_parameter("out", [DPC, 128, NRT, OCOLS], mybir.dt.float32, isOutput=True)

    st_bufs = 4 if dt_key in ("bf16", "fp16") else 2

    with TileContext(nc) as tc:
        with (
            tc.tile_pool(name="wpool", bufs=1) as wpool,
            tc.tile_pool(name="mpool", bufs=1) as mpool,
            tc.tile_pool(name="opool", bufs=st_bufs) as opool,
            tc.tile_pool(name="pspool", bufs=4, space="PSUM") as pspool,
        ):
            # warm up the PE clock (HAM) with throwaway matmuls while the
            # input DMAs are in flight, so real matmuls start at 2.4 GHz
            warm = wpool.tile([128, 512], dt_w, tag="warm", name="warm")
            nc.vector.memset(warm[:], 0.0)
            wps = pspool.tile([128, NCT, CT], mybir.dt.float32, tag="ps", name="warmps")
            for i in range(16):
                nc.tensor.matmul(
                    wps[:, 0, 0:512], warm[:, 0:128], warm[:, 0:512],
                    start=True, stop=True,
                )

            # inputs, split into k-quarters in lockstep across the two HWDGE
            # rings: the pair (wT[2s:2s+2], mats0[2s:2s+2]) lands every ~4.4us,
            # and the PE runs partial-K accumulation stages right behind the
            # arrivals (see prework below).
            wt = wpool.tile([128, KT, RWS], dt_w, tag="w", name="wt")
            m0 = mpool.tile([128, KT, OCOLS], dt_m, tag="m0", name="m0")
            for q in range(4):
                nc.sync.dma_start(out=wt[:, 2 * q:2 * q + 2, :], in_=wT[:, 2 * q:2 * q + 2, :])
                nc.scalar.dma_start(out=m0[:, 2 * q:2 * q + 2, :], in_=mats[0][:, 2 * q:2 * q + 2, :])
            mt = [m0]
            if DPC > 1:
                m1 = mpool.tile([128, KT, OCOLS], dt_m, tag="m1", name="m1")
                nc.scalar.dma_start(out=m1[:], in_=mats[1])
                mt.append(m1)

            def mms(ps, c, r, ks, ke):
                for k in range(ks, ke):
                    for ct in range(NCT):
                        nc.tensor.matmul(
                            ps[:, ct, 0:CT],
                            wt[:, k, r * 128:(r + 1) * 128],
                            mt[c][:, k, ct * CT:(ct + 1) * CT],
                            start=(k == 0),
                            stop=(k == KT - 1),
                        )

            # prework: the first 4 row-tiles of chirp 0 accumulate k-pair
            # stages chasing the quarter-DMA arrivals
            PRE = 4
            st0 = opool.tile([128, GRP, OCOLS], mybir.dt.float32, tag="st", name="st0_0")
            pre_ps = [
                pspool.tile([128, NCT, CT], mybir.dt.float32, tag="ps", name=f"ps0_{rr}")
                for rr in range(PRE)
            ]
            for stg in range(3):
                for rr in range(PRE):
                    mms(pre_ps[rr], 0, rr, 2 * stg, 2 * stg + 2)

            for c in range(DPC):
                groups = [GRP] * NG
                if c == DPC - 1:
                    groups = [GRP] * (NG - 1) + [1] * GRP
                r0 = 0
                for g, gsz in enumerate(groups):
                    if c == 0 and g == 0:
                        st = st0
                    else:
                        st = opool.tile([128, gsz, OCOLS], mybir.dt.float32, tag="st", name=f"st{c}_{g}")
                    last_grp = c == DPC - 1 and g == len(groups) - 1
                    for rr in range(gsz):
                        r = r0 + rr
                        if c == 0 and g == 0 and rr < PRE:
                            ps = pre_ps[rr]
                            mms(ps, 0, r, 6, KT)
                        else:
                            ps = pspool.tile([128, NCT, CT], mybir.dt.float32, tag="ps", name=f"ps{c}_{r}")
                            mms(ps, c, r, 0, KT)
                        if last_grp:
                            # evacuate the final tile per PSUM bank so the
                            # copy+DMA of the first half overlaps the rest
                            for ct in range(NCT):
                                nc.vector.tensor_copy(
                                    out=st[:, rr, ct * CT:(ct + 1) * CT], in_=ps[:, ct, :]
                                )
                                nc.sync.dma_start(
                                    out=out[c][:, r0 + rr:r0 + rr + 1, ct * CT:(ct + 1) * CT],
                                    in_=st[:, rr:rr + 1, ct * CT:(ct + 1) * CT],
                                )
                        else:
                            nc.vector.tensor_copy(out=st[:, rr, :], in_=ps[:].rearrange("p n x -> p (n x)"))
                    if not last_grp:
                        nc.sync.dma_start(out=out[c][:, r0:r0 + gsz, :], in_=st[:, 0:gsz, :])
                    r0 += gsz
    return nc


def _get_nc(dt_key):
    if dt_key not in _NC_CACHE:
        nc = _build_nc(dt_key)
        nc.finalize()
        _NC_CACHE[dt_key] = nc
    return _NC_CACHE[dt_key]


def _cast(arr, half):
    if half == "bf16":
        import ml_dtypes
        return arr.astype(ml_dtypes.bfloat16)
    if half == "fp16":
        return arr.astype(np.float16)
    return arr


def _dev_arrays(x, dlnf, dt_key):
    x = np.asarray(x)
    mats, win = _build_mats(np.asarray(dlnf))
    frames = np.lib.stride_tricks.sliding_window_view(x[0], K)[::HOP]  # (NW, K)
    frames = (frames * win).astype(np.float32)
    wT = np.zeros((K, RWS), np.float32)
    wT[:, :NW] = frames.T
    # partition-major: [128, KT, RWS] with [p, k, :] = wT[k*128+p, :]
    wT_pm = np.ascontiguousarray(wT.reshape(KT, 128, RWS).transpose(1, 0, 2))
    mats_pm = np.ascontiguousarray(
        mats.reshape(D, KT, 128, OCOLS).transpose(0, 2, 1, 3)
    )  # (D, 128, KT, OCOLS)
    half = dt_key if dt_key in ("bf16", "fp16") else None
    return _cast(wT_pm, half), _cast(mats_pm, half)


def kernel(x, dlnf, n_hann_splits):
    assert int(n_hann_splits) == 1
    from concourse.bass_utils import run_bass_kernel_spmd

    dt_key = DEV_DT
    nc = _get_nc(dt_key)
    wT, mats = _dev_arrays(x, dlnf, dt_key)

    core_ids = list(range(NCORES))
    in_maps = [
        {"wT": wT, "mats": np.ascontiguousarray(mats[i * DPC:(i + 1) * DPC])}
        for i in core_ids
    ]
    res = run_bass_kernel_spmd(nc, in_maps, core_ids)

    out = np.empty((D, 1, NW, K), np.complex64)
    outv = out.view(np.float32).reshape(D, 1, NW, K, 2)  # (..., K, 2) re/im
    for i in core_ids:
        dev = res.results[i]["out"]  # (DPC, 128, NRT, OCOLS) f32, partition-major
        rows = np.ascontiguousarray(dev.transpose(0, 2, 1, 3)).reshape(DPC, RWS, OCOLS)[:, :NW, :]
        sl = slice(i * DPC, (i + 1) * DPC)
        outv[sl, 0, :, 0, 0] = rows[:, :, 0]          # re0
        outv[sl, 0, :, 0, 1] = 0.0                    # im0
        outv[sl, 0, :, 1:NF - 1, :] = rows[:, :, 1:-1].reshape(DPC, NW, NF - 2, 2)
        outv[sl, 0, :, NF - 1, 0] = rows[:, :, -1]    # re512
        outv[sl, 0, :, NF - 1, 1] = 0.0               # im512
    out[:, :, :, NF:] = np.conj(out[:, :, :, 1:NF - 1][:, :, :, ::-1])
    return out
